# revision 7
# baseline (speedup 1.0000x reference)
"""ActionEncoder Trainium2 kernel (8 NeuronCores, expert-parallel).

Strategy:
- Host groups the 32768 flat actions by action_type (1=pick, 2=transport,
  3=move; type 0 rows are wait_emb and never touch the device), splits each
  group evenly across the 8 cores, and pads each per-core shard to a
  multiple of 128 (pad rows gather table row 0 and are discarded).
- Embedding tables are split on the host into bf16 hi + bf16 lo residual
  (hi+lo ~ fp24, beats fp32r precision). Each core pulls its rows with
  dma_gather(transpose=True) over 4 SWDGE queues, so gathers land directly
  feature-major; a DVE add fuses hi+lo into fp32r activations. Two fp32r
  GEMMs per expert with LeakyReLU(0.01)+bias fused on ScalarE. Output is
  written feature-major [256, C] and un-transposed/scattered on the host.
- Weights/tables are replicated per core; one SPMD NEFF for all 8 cores.
"""
import sys

import numpy as np

sys.path.insert(0, "/opt/trn_rl_repo")

import ml_dtypes

import concourse.bass as bass
import concourse.bacc as bacc
import concourse.mybir as mybir
import concourse.tile as tile
from concourse import library_config
from concourse.bass_utils import run_bass_kernel_spmd

D = 256
HID = 512
OUT = 256
NTAB = 8192
NCORES = 8
NA = 512  # max actions per compute chunk (matmul moving dim)
FP32 = mybir.dt.float32
FP32R = mybir.dt.float32r
BF16 = mybir.dt.bfloat16
INT16 = mybir.dt.int16

LAST_RESULT = None  # BassKernelResults of the most recent kernel() call

# (name, gathered tables, layer-1 K)
EXPERTS = (
    ("pick", ("agv", "from", "to", "mach"), 4 * D),
    ("trans", ("agv", "mach"), 2 * D),
    ("move", ("agv", "mach"), 2 * D),
)
TABLE_OF = {"agv": "emb_AGV", "from": "emb_operation", "to": "emb_operation", "mach": "emb_machine"}


def _chunks(c):
    """Split capacity c into chunks of <=NA, each a multiple of 128."""
    out = []
    pos = 0
    while pos < c:
        n = min(NA, c - pos)
        out.append((pos, n))
        pos += n
    return out


def _build(caps):
    """Emit the per-core BIR. caps = dict expert -> padded capacity."""
    nc = bacc.Bacc(num_swdge_queues=1)

    tabs = {}
    for tn in ("emb_operation", "emb_machine", "emb_AGV"):
        for half in ("hi", "lo"):
            tabs[(tn, half)] = nc.declare_dram_parameter(f"{tn}_{half}", [NTAB, D], BF16, isOutput=False)

    params = {}
    for name, tables, K in EXPERTS:
        c = caps[name]
        params[f"{name}_W1"] = nc.declare_dram_parameter(f"{name}_W1", [128, K // 128, HID], FP32R, isOutput=False)
        params[f"{name}_W2"] = nc.declare_dram_parameter(f"{name}_W2", [128, HID // 128, OUT], FP32R, isOutput=False)
        params[f"{name}_b1"] = nc.declare_dram_parameter(f"{name}_b1", [128, HID // 128], FP32, isOutput=False)
        params[f"{name}_b2"] = nc.declare_dram_parameter(f"{name}_b2", [128, OUT // 128], FP32, isOutput=False)
        for t in tables:
            params[f"{name}_idx_{t}"] = nc.declare_dram_parameter(f"{name}_idx_{t}", [128, c // 16], INT16, isOutput=False)
        params[f"{name}_outT"] = nc.declare_dram_parameter(f"{name}_outT", [OUT, c], FP32, isOutput=True)

    qrr = [0]  # SWDGE queue round-robin counter

    with tile.TileContext(nc) as tc:
        with (
            tc.tile_pool(name="wp", bufs=1) as wp,
            tc.tile_pool(name="xp", bufs=2) as xp,
            tc.tile_pool(name="ps", bufs=1, space="PSUM") as ps,
        ):
            nc.gpsimd.load_library(library_config.mlp)

            # --- small setup DMAs first so gathers can start immediately ---
            IDX = {}
            for name, tables, K in EXPERTS:
                c = caps[name]
                for t in tables:
                    for pos, n in _chunks(c):
                        it = wp.tile([128, n // 16], INT16, name=f"idx_{name}_{t}_{pos}")
                        nc.sync.dma_start(
                            out=it[:],
                            in_=params[f"{name}_idx_{t}"][:, pos // 16 : (pos + n) // 16],
                        )
                        IDX[(name, t, pos)] = it

            # --- weights, in first-use order ---
            W1 = {}
            W2 = {}
            B1 = {}
            B2 = {}
            for name, tables, K in EXPERTS:
                W1[name] = wp.tile([128, K // 128, HID], FP32R, name=f"w1_{name}")
                nc.sync.dma_start(out=W1[name][:], in_=params[f"{name}_W1"][:])
                B1[name] = wp.tile([128, HID // 128], FP32, name=f"b1_{name}")
                nc.sync.dma_start(out=B1[name][:], in_=params[f"{name}_b1"][:])
                W2[name] = wp.tile([128, HID // 128, OUT], FP32R, name=f"w2_{name}")
                nc.sync.dma_start(out=W2[name][:], in_=params[f"{name}_W2"][:])
                B2[name] = wp.tile([128, OUT // 128], FP32, name=f"b2_{name}")
                nc.sync.dma_start(out=B2[name][:], in_=params[f"{name}_b2"][:])

            # --- compute, chunk by chunk; gathers prefetch via tag bufs ---
            for name, tables, K in EXPERTS:
                c = caps[name]
                grp = "pick" if name == "pick" else "tm"
                for pos, n in _chunks(c):
                    # transposed hi/lo gathers: [128, 2, n] bf16 per table/half
                    gh = {}
                    for t in tables:
                        for half in ("hi", "lo"):
                            g = wp.tile(
                                [128, D // 128, n], BF16,
                                name=f"g_{name}_{t}_{half}_{pos}",
                            )
                            nc.gpsimd.dma_gather(
                                g[:],
                                tabs[(TABLE_OF[t], half)][:],
                                IDX[(name, t, pos)][:],
                                n,
                                n,
                                D,
                                transpose=True,
                                queue_num=0,
                            )
                            qrr[0] += 1
                            gh[(t, half)] = g

                    # reconstruct feature-major fp32r XT [128, K/128, n]
                    xT = xp.tile([128, K // 128, NA], FP32R, tag=f"xT_{grp}", name=f"xT_{name}")
                    for kd in range(K // 128):
                        t = tables[kd // 2]
                        h = kd % 2
                        nc.vector.tensor_add(
                            out=xT[:, kd, :n],
                            in0=gh[(t, "hi")][:, h, :],
                            in1=gh[(t, "lo")][:, h, :],
                        )

                    # layer 1: H = Prelu(X @ W1 + b1), feature-major
                    hT = xp.tile([128, HID // 128, NA], FP32R, tag="hT", name=f"hT_{name}")
                    for m in range(HID // 128):
                        p1 = ps.tile([128, NA], FP32, space="PSUM", tag="p1", bufs=2, name="p1")
                        for k in range(K // 128):
                            nc.tensor.matmul(
                                out=p1[:, :n],
                                lhsT=W1[name][:, k, m * 128 : (m + 1) * 128],
                                rhs=xT[:, k, :n],
                                start=(k == 0),
                                stop=(k == K // 128 - 1),
                            )
                        nc.scalar.activation(
                            out=hT[:, m, :n],
                            in_=p1[:, :n],
                            func=mybir.ActivationFunctionType.Prelu,
                            bias=B1[name][:, m : m + 1],
                            scale=1.0,
                            alpha=0.01,
                        )

                    # layer 2: O = H @ W2 + b2, feature-major
                    osb = xp.tile([128, OUT // 128, NA], FP32, tag="o", name=f"o_{name}")
                    for m2 in range(OUT // 128):
                        p2 = ps.tile([128, NA], FP32, space="PSUM", tag="p2", bufs=2, name="p2")
                        for k2 in range(HID // 128):
                            nc.tensor.matmul(
                                out=p2[:, :n],
                                lhsT=W2[name][:, k2, m2 * 128 : (m2 + 1) * 128],
                                rhs=hT[:, k2, :n],
                                start=(k2 == 0),
                                stop=(k2 == HID // 128 - 1),
                            )
                        nc.scalar.activation(
                            out=osb[:, m2, :n],
                            in_=p2[:, :n],
                            func=mybir.ActivationFunctionType.Identity,
                            bias=B2[name][:, m2 : m2 + 1],
                            scale=1.0,
                        )
                    for m2 in range(OUT // 128):
                        nc.sync.dma_start(
                            out=params[f"{name}_outT"][m2 * 128 : (m2 + 1) * 128, pos : pos + n],
                            in_=osb[:, m2, :n],
                        )

    nc.finalize()
    return nc


def _wrap_idx(idx, c):
    """int array [c] -> wrapped int16 [128, c//16] for dma_gather."""
    w = idx.astype(np.int16).reshape(c // 16, 16).T
    return np.ascontiguousarray(np.tile(w, (8, 1)))


def _prep_w1(w1):
    """[K, N] -> [128, K//128, N]"""
    k = w1.shape[0]
    return np.ascontiguousarray(w1.reshape(k // 128, 128, -1).transpose(1, 0, 2))


def _prep_b(b):
    """[n] -> [128, n//128]"""
    return np.ascontiguousarray(b.reshape(-1, 128).T)


def kernel(**inputs):
    global LAST_RESULT
    at = np.asarray(inputs["action_type"])
    n_act = at.shape[0]
    out = np.empty((n_act, OUT), dtype=np.float32)

    idx_in = {
        "agv": np.asarray(inputs["agv_idx"]),
        "from": np.asarray(inputs["op_from_idx"]),
        "to": np.asarray(inputs["op_to_idx"]),
        "mach": np.asarray(inputs["machine_idx"]),
    }

    rows = {}
    caps = {}
    pers = {}
    for tcode, (name, tables, K) in zip((1, 2, 3), EXPERTS):
        r = np.nonzero(at == tcode)[0]
        rows[name] = r
        pers[name] = -(-max(len(r), 1) // NCORES)  # ceil, >=1
        caps[name] = -(-pers[name] // 128) * 128

    nc = _build(caps)

    # hi/lo bf16 split of the embedding tables (shared across cores)
    tab_split = {}
    for tn in ("emb_operation", "emb_machine", "emb_AGV"):
        t = np.asarray(inputs[tn], dtype=np.float32)
        hi = t.astype(ml_dtypes.bfloat16)
        lo = (t - hi.astype(np.float32)).astype(ml_dtypes.bfloat16)
        tab_split[f"{tn}_hi"] = hi
        tab_split[f"{tn}_lo"] = lo

    in_maps = []
    for core in range(NCORES):
        m = dict(tab_split)
        for name, tables, K in EXPERTS:
            c = caps[name]
            m[f"{name}_W1"] = _prep_w1(np.asarray(inputs[f"{name}_W1"]))
            m[f"{name}_W2"] = _prep_w1(np.asarray(inputs[f"{name}_W2"]))
            m[f"{name}_b1"] = _prep_b(np.asarray(inputs[f"{name}_b1"]))
            m[f"{name}_b2"] = _prep_b(np.asarray(inputs[f"{name}_b2"]))
            r = rows[name]
            per = pers[name]
            shard = r[core * per : (core + 1) * per]
            pad = np.zeros(c, dtype=np.int64)
            pad[: len(shard)] = shard
            for t in tables:
                m[f"{name}_idx_{t}"] = _wrap_idx(idx_in[t][pad], c)
        in_maps.append(m)

    res = run_bass_kernel_spmd(nc, in_maps, list(range(NCORES)))
    LAST_RESULT = res

    # assemble
    wait_rows = np.nonzero(at == 0)[0]
    out[wait_rows] = np.asarray(inputs["wait_emb"])[None, :].astype(np.float32)
    for name, tables, K in EXPERTS:
        r = rows[name]
        if len(r) == 0:
            continue
        per = pers[name]
        full = np.concatenate(
            [res.results[core][f"{name}_outT"].T[:per] for core in range(NCORES)],
            axis=0,
        )
        out[r] = full[: len(r)]
    return out


# revision 8
# speedup vs baseline: 1.5891x; 1.5891x over previous
"""ActionEncoder Trainium2 kernel (8 NeuronCores, expert-parallel).

Strategy:
- Host groups the 32768 flat actions by action_type (1=pick, 2=transport,
  3=move; type 0 rows are wait_emb and never touch the device), splits each
  group evenly across the 8 cores, and pads each per-core shard to a
  multiple of 128 (pad rows gather table row 0 and are discarded).
- Embedding tables are split on the host into bf16 hi + bf16 lo residual
  (hi+lo ~ fp24, beats fp32r precision). Each core pulls its rows with
  dma_gather(transpose=True) over 4 SWDGE queues, so gathers land directly
  feature-major; a DVE add fuses hi+lo into fp32r activations. Two fp32r
  GEMMs per expert with LeakyReLU(0.01)+bias fused on ScalarE. Output is
  written feature-major [256, C] and un-transposed/scattered on the host.
- Weights/tables are replicated per core; one SPMD NEFF for all 8 cores.
"""
import sys

import numpy as np

sys.path.insert(0, "/opt/trn_rl_repo")

import ml_dtypes

import concourse.bass as bass
import concourse.bacc as bacc
import concourse.mybir as mybir
import concourse.tile as tile
from concourse import library_config
from concourse.bass_utils import run_bass_kernel_spmd

D = 256
HID = 512
OUT = 256
NTAB = 8192
NCORES = 8
NA = 512  # max actions per compute chunk (matmul moving dim)
FP32 = mybir.dt.float32
FP32R = mybir.dt.float32r
BF16 = mybir.dt.bfloat16
INT16 = mybir.dt.int16

LAST_RESULT = None  # BassKernelResults of the most recent kernel() call

# (name, gathered tables, layer-1 K)
EXPERTS = (
    ("pick", ("agv", "from", "to", "mach"), 4 * D),
    ("trans", ("agv", "mach"), 2 * D),
    ("move", ("agv", "mach"), 2 * D),
)
TABLE_OF = {"agv": "emb_AGV", "from": "emb_operation", "to": "emb_operation", "mach": "emb_machine"}


def _chunks(c):
    """Split capacity c into chunks of <=NA, each a multiple of 128."""
    out = []
    pos = 0
    while pos < c:
        n = min(NA, c - pos)
        out.append((pos, n))
        pos += n
    return out


def _build(caps):
    """Emit the per-core BIR. caps = dict expert -> padded capacity."""
    nc = bacc.Bacc(num_swdge_queues=2)

    tabs = {}
    for tn in ("emb_operation", "emb_machine", "emb_AGV"):
        for half in ("hi", "lo"):
            tabs[(tn, half)] = nc.declare_dram_parameter(f"{tn}_{half}", [NTAB, D], BF16, isOutput=False)

    params = {}
    for name, tables, K in EXPERTS:
        c = caps[name]
        params[f"{name}_W1"] = nc.declare_dram_parameter(f"{name}_W1", [128, K // 128, HID], FP32R, isOutput=False)
        params[f"{name}_W2"] = nc.declare_dram_parameter(f"{name}_W2", [128, HID // 128, OUT], FP32R, isOutput=False)
        params[f"{name}_b1"] = nc.declare_dram_parameter(f"{name}_b1", [128, HID // 128], FP32, isOutput=False)
        params[f"{name}_b2"] = nc.declare_dram_parameter(f"{name}_b2", [128, OUT // 128], FP32, isOutput=False)
        for t in tables:
            params[f"{name}_idx_{t}"] = nc.declare_dram_parameter(f"{name}_idx_{t}", [128, c // 16], INT16, isOutput=False)
        params[f"{name}_outT"] = nc.declare_dram_parameter(f"{name}_outT", [OUT, c], FP32, isOutput=True)

    qrr = [0]  # SWDGE queue round-robin counter

    with tile.TileContext(nc) as tc:
        with (
            tc.tile_pool(name="wp", bufs=1) as wp,
            tc.tile_pool(name="xp", bufs=2) as xp,
            tc.tile_pool(name="ps", bufs=1, space="PSUM") as ps,
        ):
            nc.gpsimd.load_library(library_config.mlp)

            # --- small setup DMAs first so gathers can start immediately ---
            IDX = {}
            for name, tables, K in EXPERTS:
                c = caps[name]
                for t in tables:
                    for pos, n in _chunks(c):
                        it = wp.tile([128, n // 16], INT16, name=f"idx_{name}_{t}_{pos}")
                        nc.sync.dma_start(
                            out=it[:],
                            in_=params[f"{name}_idx_{t}"][:, pos // 16 : (pos + n) // 16],
                        )
                        IDX[(name, t, pos)] = it

            # --- weights, in first-use order ---
            W1 = {}
            W2 = {}
            B1 = {}
            B2 = {}
            for name, tables, K in EXPERTS:
                W1[name] = wp.tile([128, K // 128, HID], FP32R, name=f"w1_{name}")
                nc.sync.dma_start(out=W1[name][:], in_=params[f"{name}_W1"][:])
                B1[name] = wp.tile([128, HID // 128], FP32, name=f"b1_{name}")
                nc.sync.dma_start(out=B1[name][:], in_=params[f"{name}_b1"][:])
                W2[name] = wp.tile([128, HID // 128, OUT], FP32R, name=f"w2_{name}")
                nc.sync.dma_start(out=W2[name][:], in_=params[f"{name}_W2"][:])
                B2[name] = wp.tile([128, OUT // 128], FP32, name=f"b2_{name}")
                nc.sync.dma_start(out=B2[name][:], in_=params[f"{name}_b2"][:])

            # --- compute, chunk by chunk; gathers prefetch via tag bufs ---
            for name, tables, K in EXPERTS:
                c = caps[name]
                grp = "pick" if name == "pick" else "tm"
                for pos, n in _chunks(c):
                    # transposed hi/lo gathers: [128, 2, n] bf16 per table/half
                    gh = {}
                    for t in tables:
                        for half in ("hi", "lo"):
                            g = wp.tile(
                                [128, D // 128, n], BF16,
                                name=f"g_{name}_{t}_{half}_{pos}",
                            )
                            nc.gpsimd.dma_gather(
                                g[:],
                                tabs[(TABLE_OF[t], half)][:],
                                IDX[(name, t, pos)][:],
                                n,
                                n,
                                D,
                                transpose=True,
                                queue_num=qrr[0] % 2,
                            )
                            qrr[0] += 1
                            gh[(t, half)] = g

                    # reconstruct feature-major fp32r XT [128, K/128, n]
                    xT = xp.tile([128, K // 128, NA], FP32R, tag=f"xT_{grp}", name=f"xT_{name}")
                    for kd in range(K // 128):
                        t = tables[kd // 2]
                        h = kd % 2
                        nc.vector.tensor_add(
                            out=xT[:, kd, :n],
                            in0=gh[(t, "hi")][:, h, :],
                            in1=gh[(t, "lo")][:, h, :],
                        )

                    # layer 1: H = Prelu(X @ W1 + b1), feature-major
                    hT = xp.tile([128, HID // 128, NA], FP32R, tag="hT", name=f"hT_{name}")
                    for m in range(HID // 128):
                        p1 = ps.tile([128, NA], FP32, space="PSUM", tag="p1", bufs=2, name="p1")
                        for k in range(K // 128):
                            nc.tensor.matmul(
                                out=p1[:, :n],
                                lhsT=W1[name][:, k, m * 128 : (m + 1) * 128],
                                rhs=xT[:, k, :n],
                                start=(k == 0),
                                stop=(k == K // 128 - 1),
                            )
                        nc.scalar.activation(
                            out=hT[:, m, :n],
                            in_=p1[:, :n],
                            func=mybir.ActivationFunctionType.Prelu,
                            bias=B1[name][:, m : m + 1],
                            scale=1.0,
                            alpha=0.01,
                        )

                    # layer 2: O = H @ W2 + b2, feature-major
                    osb = xp.tile([128, OUT // 128, NA], FP32, tag="o", name=f"o_{name}")
                    for m2 in range(OUT // 128):
                        p2 = ps.tile([128, NA], FP32, space="PSUM", tag="p2", bufs=2, name="p2")
                        for k2 in range(HID // 128):
                            nc.tensor.matmul(
                                out=p2[:, :n],
                                lhsT=W2[name][:, k2, m2 * 128 : (m2 + 1) * 128],
                                rhs=hT[:, k2, :n],
                                start=(k2 == 0),
                                stop=(k2 == HID // 128 - 1),
                            )
                        nc.scalar.activation(
                            out=osb[:, m2, :n],
                            in_=p2[:, :n],
                            func=mybir.ActivationFunctionType.Identity,
                            bias=B2[name][:, m2 : m2 + 1],
                            scale=1.0,
                        )
                    for m2 in range(OUT // 128):
                        nc.sync.dma_start(
                            out=params[f"{name}_outT"][m2 * 128 : (m2 + 1) * 128, pos : pos + n],
                            in_=osb[:, m2, :n],
                        )

    nc.finalize()
    return nc


def _wrap_idx(idx, c):
    """int array [c] -> wrapped int16 [128, c//16] for dma_gather."""
    w = idx.astype(np.int16).reshape(c // 16, 16).T
    return np.ascontiguousarray(np.tile(w, (8, 1)))


def _prep_w1(w1):
    """[K, N] -> [128, K//128, N]"""
    k = w1.shape[0]
    return np.ascontiguousarray(w1.reshape(k // 128, 128, -1).transpose(1, 0, 2))


def _prep_b(b):
    """[n] -> [128, n//128]"""
    return np.ascontiguousarray(b.reshape(-1, 128).T)


def kernel(**inputs):
    global LAST_RESULT
    at = np.asarray(inputs["action_type"])
    n_act = at.shape[0]
    out = np.empty((n_act, OUT), dtype=np.float32)

    idx_in = {
        "agv": np.asarray(inputs["agv_idx"]),
        "from": np.asarray(inputs["op_from_idx"]),
        "to": np.asarray(inputs["op_to_idx"]),
        "mach": np.asarray(inputs["machine_idx"]),
    }

    rows = {}
    caps = {}
    pers = {}
    for tcode, (name, tables, K) in zip((1, 2, 3), EXPERTS):
        r = np.nonzero(at == tcode)[0]
        rows[name] = r
        pers[name] = -(-max(len(r), 1) // NCORES)  # ceil, >=1
        caps[name] = -(-pers[name] // 128) * 128

    nc = _build(caps)

    # hi/lo bf16 split of the embedding tables (shared across cores)
    tab_split = {}
    for tn in ("emb_operation", "emb_machine", "emb_AGV"):
        t = np.asarray(inputs[tn], dtype=np.float32)
        hi = t.astype(ml_dtypes.bfloat16)
        lo = (t - hi.astype(np.float32)).astype(ml_dtypes.bfloat16)
        tab_split[f"{tn}_hi"] = hi
        tab_split[f"{tn}_lo"] = lo

    in_maps = []
    for core in range(NCORES):
        m = dict(tab_split)
        for name, tables, K in EXPERTS:
            c = caps[name]
            m[f"{name}_W1"] = _prep_w1(np.asarray(inputs[f"{name}_W1"]))
            m[f"{name}_W2"] = _prep_w1(np.asarray(inputs[f"{name}_W2"]))
            m[f"{name}_b1"] = _prep_b(np.asarray(inputs[f"{name}_b1"]))
            m[f"{name}_b2"] = _prep_b(np.asarray(inputs[f"{name}_b2"]))
            r = rows[name]
            per = pers[name]
            shard = r[core * per : (core + 1) * per]
            pad = np.zeros(c, dtype=np.int64)
            pad[: len(shard)] = shard
            for t in tables:
                m[f"{name}_idx_{t}"] = _wrap_idx(idx_in[t][pad], c)
        in_maps.append(m)

    res = run_bass_kernel_spmd(nc, in_maps, list(range(NCORES)))
    LAST_RESULT = res

    # assemble
    wait_rows = np.nonzero(at == 0)[0]
    out[wait_rows] = np.asarray(inputs["wait_emb"])[None, :].astype(np.float32)
    for name, tables, K in EXPERTS:
        r = rows[name]
        if len(r) == 0:
            continue
        per = pers[name]
        full = np.concatenate(
            [res.results[core][f"{name}_outT"].T[:per] for core in range(NCORES)],
            axis=0,
        )
        out[r] = full[: len(r)]
    return out


# revision 10
# speedup vs baseline: 1.7571x; 1.1058x over previous
"""ActionEncoder Trainium2 kernel (8 NeuronCores, expert-parallel).

Strategy:
- Host groups the 32768 flat actions by action_type (1=pick, 2=transport,
  3=move; type 0 rows are wait_emb and never touch the device), splits each
  group evenly across the 8 cores, and pads each per-core shard to a
  multiple of 128 (pad rows gather table row 0 and are discarded).
- Embedding tables are split on the host into bf16 hi + bf16 lo residual
  (hi+lo ~ fp24, beats fp32r precision). Each core pulls its rows with
  dma_gather(transpose=True) over 4 SWDGE queues, so gathers land directly
  feature-major; a DVE add fuses hi+lo into fp32r activations. Two fp32r
  GEMMs per expert with LeakyReLU(0.01)+bias fused on ScalarE. Output is
  written feature-major [256, C] and un-transposed/scattered on the host.
- Weights/tables are replicated per core; one SPMD NEFF for all 8 cores.
"""
import sys

import numpy as np

sys.path.insert(0, "/opt/trn_rl_repo")

import ml_dtypes

import concourse.bass as bass
import concourse.bacc as bacc
import concourse.mybir as mybir
import concourse.tile as tile
from concourse import library_config
from concourse.bass_utils import run_bass_kernel_spmd

D = 256
HID = 512
OUT = 256
NTAB = 8192
NCORES = 8
NA = 512  # max actions per compute chunk (matmul moving dim)
FP32 = mybir.dt.float32
FP32R = mybir.dt.float32r
BF16 = mybir.dt.bfloat16
INT16 = mybir.dt.int16

LAST_RESULT = None  # BassKernelResults of the most recent kernel() call

# (name, gathered tables, layer-1 K)
EXPERTS = (
    ("pick", ("agv", "from", "to", "mach"), 4 * D),
    ("trans", ("agv", "mach"), 2 * D),
    ("move", ("agv", "mach"), 2 * D),
)
TABLE_OF = {"agv": "emb_AGV", "from": "emb_operation", "to": "emb_operation", "mach": "emb_machine"}


def _chunks(c):
    """Split capacity c into chunks of <=NA, each a multiple of 128."""
    out = []
    pos = 0
    while pos < c:
        n = min(NA, c - pos)
        out.append((pos, n))
        pos += n
    return out


def _build(caps):
    """Emit the per-core BIR. caps = dict expert -> padded capacity."""
    nc = bacc.Bacc(num_swdge_queues=4)

    tabs = {}
    for tn in ("emb_operation", "emb_machine", "emb_AGV"):
        tabs[tn] = nc.declare_dram_parameter(f"{tn}_cat", [NTAB, 2 * D], BF16, isOutput=False)

    params = {}
    for name, tables, K in EXPERTS:
        c = caps[name]
        params[f"{name}_W1"] = nc.declare_dram_parameter(f"{name}_W1", [128, K // 128, HID], FP32R, isOutput=False)
        params[f"{name}_W2"] = nc.declare_dram_parameter(f"{name}_W2", [128, HID // 128, OUT], FP32R, isOutput=False)
        params[f"{name}_b1"] = nc.declare_dram_parameter(f"{name}_b1", [128, HID // 128], FP32, isOutput=False)
        params[f"{name}_b2"] = nc.declare_dram_parameter(f"{name}_b2", [128, OUT // 128], FP32, isOutput=False)
        for t in tables:
            params[f"{name}_idx_{t}"] = nc.declare_dram_parameter(f"{name}_idx_{t}", [128, c // 16], INT16, isOutput=False)
        params[f"{name}_outT"] = nc.declare_dram_parameter(f"{name}_outT", [OUT, c], FP32, isOutput=True)

    qrr = [0]  # SWDGE queue round-robin counter

    with tile.TileContext(nc) as tc:
        with (
            tc.tile_pool(name="wp", bufs=1) as wp,
            tc.tile_pool(name="xp", bufs=2) as xp,
            tc.tile_pool(name="ps", bufs=1, space="PSUM") as ps,
        ):
            nc.gpsimd.load_library(library_config.mlp)

            # --- small setup DMAs first so gathers can start immediately ---
            IDX = {}
            for name, tables, K in EXPERTS:
                c = caps[name]
                for t in tables:
                    for pos, n in _chunks(c):
                        it = wp.tile([128, n // 16], INT16, name=f"idx_{name}_{t}_{pos}")
                        nc.sync.dma_start(
                            out=it[:],
                            in_=params[f"{name}_idx_{t}"][:, pos // 16 : (pos + n) // 16],
                        )
                        IDX[(name, t, pos)] = it

            # --- weights, in first-use order ---
            W1 = {}
            W2 = {}
            B1 = {}
            B2 = {}
            for name, tables, K in EXPERTS:
                W1[name] = wp.tile([128, K // 128, HID], FP32R, name=f"w1_{name}")
                nc.sync.dma_start(out=W1[name][:], in_=params[f"{name}_W1"][:])
                B1[name] = wp.tile([128, HID // 128], FP32, name=f"b1_{name}")
                nc.sync.dma_start(out=B1[name][:], in_=params[f"{name}_b1"][:])
                W2[name] = wp.tile([128, HID // 128, OUT], FP32R, name=f"w2_{name}")
                nc.sync.dma_start(out=W2[name][:], in_=params[f"{name}_W2"][:])
                B2[name] = wp.tile([128, OUT // 128], FP32, name=f"b2_{name}")
                nc.sync.dma_start(out=B2[name][:], in_=params[f"{name}_b2"][:])

            # --- compute, chunk by chunk; gathers prefetch via tag bufs ---
            for name, tables, K in EXPERTS:
                c = caps[name]
                grp = "pick" if name == "pick" else "tm"
                for pos, n in _chunks(c):
                    # transposed hi/lo gathers: [128, 2, n] bf16 per table/half
                    gh = {}
                    for t in tables:
                        g = wp.tile(
                            [128, 2 * D // 128, n], BF16,
                            name=f"g_{name}_{t}_{pos}",
                        )
                        nc.gpsimd.dma_gather(
                            g[:],
                            tabs[TABLE_OF[t]][:],
                            IDX[(name, t, pos)][:],
                            n,
                            n,
                            2 * D,
                            transpose=True,
                            queue_num=qrr[0] % 4,
                        )
                        qrr[0] += 1
                        gh[t] = g

                    # reconstruct feature-major fp32r XT [128, K/128, n]
                    xT = xp.tile([128, K // 128, NA], FP32R, tag=f"xT_{grp}", name=f"xT_{name}")
                    for kd in range(K // 128):
                        t = tables[kd // 2]
                        h = kd % 2
                        nc.vector.tensor_add(
                            out=xT[:, kd, :n],
                            in0=gh[t][:, h, :],
                            in1=gh[t][:, h + 2, :],
                        )

                    # layer 1: H = Prelu(X @ W1 + b1), feature-major
                    hT = xp.tile([128, HID // 128, NA], FP32R, tag="hT", name=f"hT_{name}")
                    for m in range(HID // 128):
                        p1 = ps.tile([128, NA], FP32, space="PSUM", tag="p1", bufs=2, name="p1")
                        for k in range(K // 128):
                            nc.tensor.matmul(
                                out=p1[:, :n],
                                lhsT=W1[name][:, k, m * 128 : (m + 1) * 128],
                                rhs=xT[:, k, :n],
                                start=(k == 0),
                                stop=(k == K // 128 - 1),
                            )
                        nc.scalar.activation(
                            out=hT[:, m, :n],
                            in_=p1[:, :n],
                            func=mybir.ActivationFunctionType.Prelu,
                            bias=B1[name][:, m : m + 1],
                            scale=1.0,
                            alpha=0.01,
                        )

                    # layer 2: O = H @ W2 + b2, feature-major
                    osb = xp.tile([128, OUT // 128, NA], FP32, tag="o", name=f"o_{name}")
                    for m2 in range(OUT // 128):
                        p2 = ps.tile([128, NA], FP32, space="PSUM", tag="p2", bufs=2, name="p2")
                        for k2 in range(HID // 128):
                            nc.tensor.matmul(
                                out=p2[:, :n],
                                lhsT=W2[name][:, k2, m2 * 128 : (m2 + 1) * 128],
                                rhs=hT[:, k2, :n],
                                start=(k2 == 0),
                                stop=(k2 == HID // 128 - 1),
                            )
                        nc.scalar.activation(
                            out=osb[:, m2, :n],
                            in_=p2[:, :n],
                            func=mybir.ActivationFunctionType.Identity,
                            bias=B2[name][:, m2 : m2 + 1],
                            scale=1.0,
                        )
                    for m2 in range(OUT // 128):
                        nc.sync.dma_start(
                            out=params[f"{name}_outT"][m2 * 128 : (m2 + 1) * 128, pos : pos + n],
                            in_=osb[:, m2, :n],
                        )

    nc.finalize()
    return nc


def _wrap_idx(idx, c):
    """int array [c] -> wrapped int16 [128, c//16] for dma_gather."""
    w = idx.astype(np.int16).reshape(c // 16, 16).T
    return np.ascontiguousarray(np.tile(w, (8, 1)))


def _prep_w1(w1):
    """[K, N] -> [128, K//128, N]"""
    k = w1.shape[0]
    return np.ascontiguousarray(w1.reshape(k // 128, 128, -1).transpose(1, 0, 2))


def _prep_b(b):
    """[n] -> [128, n//128]"""
    return np.ascontiguousarray(b.reshape(-1, 128).T)


def kernel(**inputs):
    global LAST_RESULT
    at = np.asarray(inputs["action_type"])
    n_act = at.shape[0]
    out = np.empty((n_act, OUT), dtype=np.float32)

    idx_in = {
        "agv": np.asarray(inputs["agv_idx"]),
        "from": np.asarray(inputs["op_from_idx"]),
        "to": np.asarray(inputs["op_to_idx"]),
        "mach": np.asarray(inputs["machine_idx"]),
    }

    rows = {}
    caps = {}
    pers = {}
    for tcode, (name, tables, K) in zip((1, 2, 3), EXPERTS):
        r = np.nonzero(at == tcode)[0]
        rows[name] = r
        pers[name] = -(-max(len(r), 1) // NCORES)  # ceil, >=1
        caps[name] = -(-pers[name] // 128) * 128

    nc = _build(caps)

    # hi/lo bf16 split of the embedding tables (shared across cores)
    tab_split = {}
    for tn in ("emb_operation", "emb_machine", "emb_AGV"):
        t = np.asarray(inputs[tn], dtype=np.float32)
        hi = t.astype(ml_dtypes.bfloat16)
        lo = (t - hi.astype(np.float32)).astype(ml_dtypes.bfloat16)
        tab_split[f"{tn}_cat"] = np.ascontiguousarray(np.concatenate([hi, lo], axis=1))

    in_maps = []
    for core in range(NCORES):
        m = dict(tab_split)
        for name, tables, K in EXPERTS:
            c = caps[name]
            m[f"{name}_W1"] = _prep_w1(np.asarray(inputs[f"{name}_W1"]))
            m[f"{name}_W2"] = _prep_w1(np.asarray(inputs[f"{name}_W2"]))
            m[f"{name}_b1"] = _prep_b(np.asarray(inputs[f"{name}_b1"]))
            m[f"{name}_b2"] = _prep_b(np.asarray(inputs[f"{name}_b2"]))
            r = rows[name]
            per = pers[name]
            shard = r[core * per : (core + 1) * per]
            pad = np.zeros(c, dtype=np.int64)
            pad[: len(shard)] = shard
            for t in tables:
                m[f"{name}_idx_{t}"] = _wrap_idx(idx_in[t][pad], c)
        in_maps.append(m)

    res = run_bass_kernel_spmd(nc, in_maps, list(range(NCORES)))
    LAST_RESULT = res

    # assemble
    wait_rows = np.nonzero(at == 0)[0]
    out[wait_rows] = np.asarray(inputs["wait_emb"])[None, :].astype(np.float32)
    for name, tables, K in EXPERTS:
        r = rows[name]
        if len(r) == 0:
            continue
        per = pers[name]
        full = np.concatenate(
            [res.results[core][f"{name}_outT"].T[:per] for core in range(NCORES)],
            axis=0,
        )
        out[r] = full[: len(r)]
    return out


# revision 12
# speedup vs baseline: 1.9636x; 1.1175x over previous
"""ActionEncoder Trainium2 kernel (8 NeuronCores, expert-parallel).

Strategy:
- Host groups the 32768 flat actions by action_type (1=pick, 2=transport,
  3=move; type 0 rows are wait_emb and never touch the device), splits each
  group evenly across the 8 cores, and pads each per-core shard to a
  multiple of 128 (pad rows gather table row 0 and are discarded).
- Embedding tables are split on the host into bf16 hi + bf16 lo residual
  (hi+lo ~ fp24, beats fp32r precision). Each core pulls its rows with
  dma_gather(transpose=True) over 4 SWDGE queues, so gathers land directly
  feature-major; a DVE add fuses hi+lo into fp32r activations. Two fp32r
  GEMMs per expert with LeakyReLU(0.01)+bias fused on ScalarE. Output is
  written feature-major [256, C] and un-transposed/scattered on the host.
- Weights/tables are replicated per core; one SPMD NEFF for all 8 cores.
"""
import sys

import numpy as np

sys.path.insert(0, "/opt/trn_rl_repo")

import ml_dtypes

import concourse.bass as bass
import concourse.bacc as bacc
import concourse.mybir as mybir
import concourse.tile as tile
from concourse import library_config
from concourse.bass_utils import run_bass_kernel_spmd

D = 256
HID = 512
OUT = 256
NTAB = 8192
NCORES = 8
NA = 512  # max actions per compute chunk (matmul moving dim)
FP32 = mybir.dt.float32
FP32R = mybir.dt.float32r
BF16 = mybir.dt.bfloat16
INT16 = mybir.dt.int16

LAST_RESULT = None  # BassKernelResults of the most recent kernel() call

# (name, gathered tables, layer-1 K)
EXPERTS = (
    ("pick", ("agv", "from", "to", "mach"), 4 * D),
    ("trans", ("agv", "mach"), 2 * D),
    ("move", ("agv", "mach"), 2 * D),
)
TABLE_OF = {"agv": "emb_AGV", "from": "emb_operation", "to": "emb_operation", "mach": "emb_machine"}


def _chunks(c):
    """Split capacity c into chunks of <=NA, each a multiple of 128."""
    out = []
    pos = 0
    while pos < c:
        n = min(NA, c - pos)
        out.append((pos, n))
        pos += n
    return out


def _build(caps):
    """Emit the per-core BIR. caps = dict expert -> padded capacity."""
    nc = bacc.Bacc(num_swdge_queues=4)

    tabs = {}
    for tn in ("emb_operation", "emb_machine", "emb_AGV"):
        tabs[tn] = nc.declare_dram_parameter(f"{tn}_cat", [NTAB, 2 * D], BF16, isOutput=False)

    params = {}
    for name, tables, K in EXPERTS:
        c = caps[name]
        params[f"{name}_W1"] = nc.declare_dram_parameter(f"{name}_W1", [128, K // 128, HID], FP32R, isOutput=False)
        params[f"{name}_W2"] = nc.declare_dram_parameter(f"{name}_W2", [128, HID // 128, OUT], FP32R, isOutput=False)
        params[f"{name}_b1"] = nc.declare_dram_parameter(f"{name}_b1", [128, HID // 128], FP32, isOutput=False)
        params[f"{name}_b2"] = nc.declare_dram_parameter(f"{name}_b2", [128, OUT // 128], FP32, isOutput=False)
        params[f"{name}_outT"] = nc.declare_dram_parameter(f"{name}_outT", [OUT, c], FP32, isOutput=True)

    seg_off = {}
    off = 0
    for name, tables, K in EXPERTS:
        for t in tables:
            seg_off[(name, t)] = off
            off += caps[name] // 16
    params["idx_all"] = nc.declare_dram_parameter("idx_all", [128, off], INT16, isOutput=False)
    params["warm_sum"] = nc.declare_dram_parameter("warm_sum", [128, 1], FP32, isOutput=True)

    qrr = [0]  # SWDGE queue round-robin counter

    with tile.TileContext(nc) as tc:
        with (
            tc.tile_pool(name="wp", bufs=1) as wp,
            tc.tile_pool(name="xp", bufs=2) as xp,
            tc.tile_pool(name="ps", bufs=1, space="PSUM") as ps,
        ):
            nc.gpsimd.load_library(library_config.mlp)

            # --- one idx DMA, then 4 warmup gathers to open the queues ---
            idx_all = wp.tile([128, off], INT16, name="idx_all")
            nc.sync.dma_start(out=idx_all[:], in_=params["idx_all"][:])
            wsum = wp.tile([128, 1], FP32, name="wsum")
            nc.gpsimd.memset(wsum[:], 0.0)
            nc.sync.dma_start(out=params["warm_sum"][:], in_=wsum[:])

            # --- weights, in first-use order ---
            W1 = {}
            W2 = {}
            B1 = {}
            B2 = {}
            for name, tables, K in EXPERTS:
                W1[name] = wp.tile([128, K // 128, HID], FP32R, name=f"w1_{name}")
                nc.sync.dma_start(out=W1[name][:], in_=params[f"{name}_W1"][:])
                B1[name] = wp.tile([128, HID // 128], FP32, name=f"b1_{name}")
                nc.sync.dma_start(out=B1[name][:], in_=params[f"{name}_b1"][:])
                W2[name] = wp.tile([128, HID // 128, OUT], FP32R, name=f"w2_{name}")
                nc.sync.dma_start(out=W2[name][:], in_=params[f"{name}_W2"][:])
                B2[name] = wp.tile([128, OUT // 128], FP32, name=f"b2_{name}")
                nc.sync.dma_start(out=B2[name][:], in_=params[f"{name}_b2"][:])

            # --- compute, chunk by chunk; gathers prefetch via tag bufs ---
            for name, tables, K in EXPERTS:
                c = caps[name]
                grp = "pick" if name == "pick" else "tm"
                for pos, n in _chunks(c):
                    # transposed hi/lo gathers: [128, 2, n] bf16 per table/half
                    gh = {}
                    for t in tables:
                        g = wp.tile(
                            [128, 2 * D // 128, n], BF16,
                            name=f"g_{name}_{t}_{pos}",
                        )
                        so = seg_off[(name, t)]
                        nc.gpsimd.dma_gather(
                            g[:],
                            tabs[TABLE_OF[t]][:],
                            idx_all[:, so + pos // 16 : so + (pos + n) // 16],
                            n,
                            n,
                            2 * D,
                            transpose=True,
                            queue_num=qrr[0] % 4,
                        )
                        qrr[0] += 1
                        gh[t] = g

                    # reconstruct feature-major fp32r XT [128, K/128, n]
                    xT = xp.tile([128, K // 128, NA], FP32R, tag=f"xT_{grp}", name=f"xT_{name}")
                    for kd in range(K // 128):
                        t = tables[kd // 2]
                        h = kd % 2
                        nc.vector.tensor_add(
                            out=xT[:, kd, :n],
                            in0=gh[t][:, h, :],
                            in1=gh[t][:, h + 2, :],
                        )

                    # layer 1: H = Prelu(X @ W1 + b1), feature-major
                    hT = xp.tile([128, HID // 128, NA], FP32R, tag="hT", name=f"hT_{name}")
                    for m in range(HID // 128):
                        p1 = ps.tile([128, NA], FP32, space="PSUM", tag="p1", bufs=2, name="p1")
                        for k in range(K // 128):
                            nc.tensor.matmul(
                                out=p1[:, :n],
                                lhsT=W1[name][:, k, m * 128 : (m + 1) * 128],
                                rhs=xT[:, k, :n],
                                start=(k == 0),
                                stop=(k == K // 128 - 1),
                            )
                        nc.scalar.activation(
                            out=hT[:, m, :n],
                            in_=p1[:, :n],
                            func=mybir.ActivationFunctionType.Prelu,
                            bias=B1[name][:, m : m + 1],
                            scale=1.0,
                            alpha=0.01,
                        )

                    # layer 2: O = H @ W2 + b2, feature-major
                    osb = xp.tile([128, OUT // 128, NA], FP32, tag="o", name=f"o_{name}")
                    for m2 in range(OUT // 128):
                        p2 = ps.tile([128, NA], FP32, space="PSUM", tag="p2", bufs=2, name="p2")
                        for k2 in range(HID // 128):
                            nc.tensor.matmul(
                                out=p2[:, :n],
                                lhsT=W2[name][:, k2, m2 * 128 : (m2 + 1) * 128],
                                rhs=hT[:, k2, :n],
                                start=(k2 == 0),
                                stop=(k2 == HID // 128 - 1),
                            )
                        nc.scalar.activation(
                            out=osb[:, m2, :n],
                            in_=p2[:, :n],
                            func=mybir.ActivationFunctionType.Identity,
                            bias=B2[name][:, m2 : m2 + 1],
                            scale=1.0,
                        )
                    for m2 in range(OUT // 128):
                        nc.sync.dma_start(
                            out=params[f"{name}_outT"][m2 * 128 : (m2 + 1) * 128, pos : pos + n],
                            in_=osb[:, m2, :n],
                        )

    nc.finalize()
    return nc


def _wrap_idx(idx, c):
    """int array [c] -> wrapped int16 [128, c//16] for dma_gather."""
    w = idx.astype(np.int16).reshape(c // 16, 16).T
    return np.ascontiguousarray(np.tile(w, (8, 1)))


def _prep_w1(w1):
    """[K, N] -> [128, K//128, N]"""
    k = w1.shape[0]
    return np.ascontiguousarray(w1.reshape(k // 128, 128, -1).transpose(1, 0, 2))


def _prep_b(b):
    """[n] -> [128, n//128]"""
    return np.ascontiguousarray(b.reshape(-1, 128).T)


def kernel(**inputs):
    global LAST_RESULT
    at = np.asarray(inputs["action_type"])
    n_act = at.shape[0]
    out = np.empty((n_act, OUT), dtype=np.float32)

    idx_in = {
        "agv": np.asarray(inputs["agv_idx"]),
        "from": np.asarray(inputs["op_from_idx"]),
        "to": np.asarray(inputs["op_to_idx"]),
        "mach": np.asarray(inputs["machine_idx"]),
    }

    rows = {}
    caps = {}
    pers = {}
    for tcode, (name, tables, K) in zip((1, 2, 3), EXPERTS):
        r = np.nonzero(at == tcode)[0]
        rows[name] = r
        pers[name] = -(-max(len(r), 1) // NCORES)  # ceil, >=1
        caps[name] = -(-pers[name] // 128) * 128

    nc = _build(caps)

    # hi/lo bf16 split of the embedding tables (shared across cores)
    tab_split = {}
    for tn in ("emb_operation", "emb_machine", "emb_AGV"):
        t = np.asarray(inputs[tn], dtype=np.float32)
        hi = t.astype(ml_dtypes.bfloat16)
        lo = (t - hi.astype(np.float32)).astype(ml_dtypes.bfloat16)
        tab_split[f"{tn}_cat"] = np.ascontiguousarray(np.concatenate([hi, lo], axis=1))

    in_maps = []
    for core in range(NCORES):
        m = dict(tab_split)
        segs = {}
        for name, tables, K in EXPERTS:
            c = caps[name]
            m[f"{name}_W1"] = _prep_w1(np.asarray(inputs[f"{name}_W1"]))
            m[f"{name}_W2"] = _prep_w1(np.asarray(inputs[f"{name}_W2"]))
            m[f"{name}_b1"] = _prep_b(np.asarray(inputs[f"{name}_b1"]))
            m[f"{name}_b2"] = _prep_b(np.asarray(inputs[f"{name}_b2"]))
            r = rows[name]
            per = pers[name]
            shard = r[core * per : (core + 1) * per]
            pad = np.zeros(c, dtype=np.int64)
            pad[: len(shard)] = shard
            for t in tables:
                segs[(name, t)] = _wrap_idx(idx_in[t][pad], c)
        m["idx_all"] = np.concatenate(
            [segs[(name, t)] for name, tables, K in EXPERTS for t in tables], axis=1
        )
        in_maps.append(m)

    res = run_bass_kernel_spmd(nc, in_maps, list(range(NCORES)))
    LAST_RESULT = res

    # assemble
    wait_rows = np.nonzero(at == 0)[0]
    out[wait_rows] = np.asarray(inputs["wait_emb"])[None, :].astype(np.float32)
    for name, tables, K in EXPERTS:
        r = rows[name]
        if len(r) == 0:
            continue
        per = pers[name]
        full = np.concatenate(
            [res.results[core][f"{name}_outT"].T[:per] for core in range(NCORES)],
            axis=0,
        )
        out[r] = full[: len(r)]
    return out
